# revision 27
# baseline (speedup 1.0000x reference)
"""Causal multi-head self-attention (RoPE) for Trainium2, distributed over 8 NeuronCores.

Sharding strategy (tensor-parallel over heads x data-parallel over batch):
  core c handles batch b = c // 2 and head-group g = c % 2 (8 of 16 heads).
  Each core computes q/k/v projections for its 8 heads on its batch, RoPE,
  block-causal flash-style attention, and the output projection against its
  512 rows of wo -- producing a partial [S, D] output.  The host-side gather
  sums the two partials per batch (the tensor-parallel reduce) and stacks
  batches to the full [B, S, D] output.

Device design notes:
  - All matmuls run with the contraction dim on partitions, so the host feeds
    x and the weights pre-transposed (pure layout work, no host FLOPs).
  - Compute dtype on the tensor engine is bf16 (fp32 PSUM accumulation);
    RoPE tables/softmax normalization stay fp32.
  - q/k are kept transposed [head_dim, S]; RoPE pairs are rotated with a
    32-lane stream_shuffle (partition pair-swap) + sign-folded sin table.
  - Scores are computed transposed [keys, queries] so the exp'ed
    probabilities feed the PV matmul as the moving operand, no transposes.
  - The softmax normalizer comes from a ones-column appended to v (row 64 of
    the PV accumulator); no row-max subtraction is needed because scores are
    ~N(0,1) bounded, so exp cannot overflow fp32.
"""

import math
import sys

import numpy as np

if "/opt/trn_rl_repo" not in sys.path:
    sys.path.insert(0, "/opt/trn_rl_repo")

import contextlib

import concourse.bacc as bacc
import concourse.tile as tile
from concourse import mybir
from concourse.bass_interp import get_hw_module
from concourse.bass_utils import run_bass_kernel_spmd

F32 = mybir.dt.float32
BF16 = mybir.dt.bfloat16
I32 = mybir.dt.int32

B, S, D = 4, 2048, 1024
H, DH = 16, 64
GD = 512           # head dims per core (8 heads)
THETA = 10000.0
SWAP_MASK = [i ^ 1 for i in range(32)]


def _build_program(dbg=False):
    nc = bacc.Bacc("TRN2", target_bir_lowering=False, debug=False,
                   enable_asserts=False, num_devices=8)

    xT = nc.dram_tensor("xT", [D, S], BF16, kind="ExternalInput").ap()
    wqT = nc.dram_tensor("wqT", [D, GD], BF16, kind="ExternalInput").ap()
    wkT = nc.dram_tensor("wkT", [D, GD], BF16, kind="ExternalInput").ap()
    wvT = nc.dram_tensor("wvT", [D, GD], BF16, kind="ExternalInput").ap()
    woT = nc.dram_tensor("woT", [GD, D], BF16, kind="ExternalInput").ap()
    posd = nc.dram_tensor("pos", [1, S], I32, kind="ExternalInput").ap()
    invfd = nc.dram_tensor("invf", [128, 1], F32, kind="ExternalInput").ap()
    altd = nc.dram_tensor("altsign", [128, 1], F32, kind="ExternalInput").ap()
    maskd = nc.dram_tensor("mask01", [128, 4 * 512], F32, kind="ExternalInput").ap()
    outp = nc.dram_tensor("outp", [S, D], F32, kind="ExternalOutput").ap()

    dbg_aps = None
    if dbg:
        dbg_aps = {
            "dq0": nc.dram_tensor("dq0", [128, S], BF16, kind="ExternalOutput").ap(),
            "dk0": nc.dram_tensor("dk0", [128, S], BF16, kind="ExternalOutput").ap(),
            "dv0": nc.dram_tensor("dv0", [128, 520], BF16, kind="ExternalOutput").ap(),
            "do0": nc.dram_tensor("do0", [128, S], BF16, kind="ExternalOutput").ap(),
            "do1": nc.dram_tensor("do1", [128, S], BF16, kind="ExternalOutput").ap(),
            "do2": nc.dram_tensor("do2", [128, S], BF16, kind="ExternalOutput").ap(),
            "do3": nc.dram_tensor("do3", [128, S], BF16, kind="ExternalOutput").ap(),
            "dsin": nc.dram_tensor("dsin", [128, S], F32, kind="ExternalOutput").ap(),
            "dposb": nc.dram_tensor("dposb", [128, S], F32, kind="ExternalOutput").ap(),
            "dtt": nc.dram_tensor("dtt", [128, S], F32, kind="ExternalOutput").ap(),
            "dtf": nc.dram_tensor("dtf", [128, S], F32, kind="ExternalOutput").ap(),
            "drr": nc.dram_tensor("drr", [128, S], F32, kind="ExternalOutput").ap(),
            "dcos": nc.dram_tensor("dcos", [128, S], F32, kind="ExternalOutput").ap(),
            "dpt": nc.dram_tensor("dpt", [128, 1024], BF16, kind="ExternalOutput").ap(),
            "dps2": nc.dram_tensor("dps2", [128, 1024], F32, kind="ExternalOutput").ap(),
            "dpo0": nc.dram_tensor("dpo0", [128, 512], F32, kind="ExternalOutput").ap(),
        }
    with tile.TileContext(nc) as tc:
        _body(tc, nc, xT, wqT, wkT, wvT, woT, posd, invfd, altd, maskd, outp,
              dbg_aps=dbg_aps)
    nc.compile()
    return nc


def _body(tc, nc, xT, wqT, wkT, wvT, woT, posd, invfd, altd, maskd, outp,
          dbg_aps=None):
    ctx = contextlib.ExitStack()

    # ---- rope tables / mask: built in a scratch pool, results persist ------
    singles = ctx.enter_context(tc.tile_pool(name="singles", bufs=1))
    maskb = singles.tile([128, 2048], BF16, tag="maskb")
    sinI = singles.tile([128, S], F32, tag="sinI")
    cosI = singles.tile([128, S], F32, tag="cosI")
    sinA = singles.tile([128, S], F32, tag="sinA")

    with tc.tile_pool(name="ropebuild", bufs=1) as rb:
        maskf = rb.tile([128, 2048], F32, tag="maskf")
        nc.sync.dma_start(out=maskf, in_=maskd)
        nc.vector.tensor_copy(out=maskb, in_=maskf)

        invf = rb.tile([128, 1], F32, tag="invf")
        nc.sync.dma_start(out=invf, in_=invfd)
        altsign = rb.tile([128, 1], F32, tag="altsign")
        nc.sync.dma_start(out=altsign, in_=altd)
        posi = rb.tile([1, S], I32, tag="posi")
        nc.sync.dma_start(out=posi, in_=posd)
        posf = rb.tile([1, S], F32, tag="posf")
        nc.vector.tensor_copy(out=posf, in_=posi)
        posb = rb.tile([128, S], F32, tag="posb")
        nc.gpsimd.partition_broadcast(posb, posf, 128)

        # t = pos * invf / (2 pi); r = t mod-centered into [-0.5, 0.5].
        # The f32->i32 convert rounding mode differs between sim (trunc) and
        # HW (nearest), so reduce with a convert + compare-and-correct that is
        # valid under either mode (any integer k with |t-k| <= 1 is fixed up).
        tt = rb.tile([128, S], F32, tag="tt")
        nc.vector.tensor_scalar(tt, posb, invf, float(1.0 / (2 * math.pi)),
                                mybir.AluOpType.mult, mybir.AluOpType.mult)

        _cf_n = [0]

        def centered_frac(dst, src, extra):
            # dst = src + extra - k, corrected into [-0.5, 0.5]
            _cf_n[0] += 1
            n = _cf_n[0]
            a = rb.tile([128, S], F32, tag="cf_s0", name=f"cf_a{n}")
            nc.vector.tensor_scalar(a, src, float(extra), None, mybir.AluOpType.add)
            ki = rb.tile([128, S], I32, tag="cf_s1", name=f"cf_ki{n}")
            nc.vector.tensor_copy(out=ki, in_=a)
            kf = rb.tile([128, S], F32, tag="cf_s2", name=f"cf_kf{n}")
            nc.vector.tensor_copy(out=kf, in_=ki)
            r0 = rb.tile([128, S], F32, tag="cf_s1", name=f"cf_r0{n}")
            nc.vector.tensor_sub(r0, a, kf)
            m1 = rb.tile([128, S], F32, tag="cf_s2", name=f"cf_m1{n}")
            nc.vector.tensor_scalar(m1, r0, 0.5, None, mybir.AluOpType.is_gt)
            r1 = rb.tile([128, S], F32, tag="cf_s0", name=f"cf_r1{n}")
            nc.vector.tensor_sub(r1, r0, m1)
            m2 = rb.tile([128, S], F32, tag="cf_s1", name=f"cf_m2{n}")
            nc.vector.tensor_scalar(m2, r1, -0.5, None, mybir.AluOpType.is_lt)
            nc.vector.tensor_add(dst, r1, m2)

        rr = rb.tile([128, S], F32, tag="rr")
        centered_frac(rr, tt, 0.0)
        if dbg_aps is not None:
            nc.sync.dma_start(out=dbg_aps["dposb"], in_=posb[:])
            nc.sync.dma_start(out=dbg_aps["dtt"], in_=tt[:])
            nc.sync.dma_start(out=dbg_aps["dtf"], in_=tt[:])
            nc.sync.dma_start(out=dbg_aps["drr"], in_=rr[:])
        nc.scalar.activation(sinI, rr, mybir.ActivationFunctionType.Sin,
                             scale=float(2 * math.pi))
        # cos(2 pi t) = sin(2 pi centered_frac(t + 0.25))
        rr2 = rb.tile([128, S], F32, tag="rr", name="rr2")
        centered_frac(rr2, tt, 0.25)
        nc.scalar.activation(cosI, rr2, mybir.ActivationFunctionType.Sin,
                             scale=float(2 * math.pi))
        nc.vector.tensor_scalar(sinA, sinI, altsign, None, mybir.AluOpType.mult)

    # ---- weights ------------------------------------------------------------
    wq_sb = [singles.tile([128, GD], BF16, tag=f"wq{i}", name=f"wq{i}") for i in range(8)]
    wk_sb = [singles.tile([128, GD], BF16, tag=f"wk{i}", name=f"wk{i}") for i in range(8)]
    wv_sb = [singles.tile([128, GD], BF16, tag=f"wv{i}", name=f"wv{i}") for i in range(8)]
    for i in range(8):
        nc.sync.dma_start(out=wq_sb[i], in_=wqT[i * 128:(i + 1) * 128, :])
        nc.sync.dma_start(out=wk_sb[i], in_=wkT[i * 128:(i + 1) * 128, :])
        nc.sync.dma_start(out=wv_sb[i], in_=wvT[i * 128:(i + 1) * 128, :])
    wo_sb = [singles.tile([128, D], BF16, tag=f"wo{i}", name=f"wo{i}") for i in range(4)]
    for i in range(4):
        nc.sync.dma_start(out=wo_sb[i], in_=woT[i * 128:(i + 1) * 128, :])

    # ---- persistent activations --------------------------------------------
    qT = [singles.tile([128, S], BF16, tag=f"qT{i}", name=f"qT{i}") for i in range(4)]
    kT = [singles.tile([128, S], BF16, tag=f"kT{i}", name=f"kT{i}") for i in range(4)]
    vt = [singles.tile([128, 8 * 65], BF16, tag=f"v{i}", name=f"v{i}") for i in range(16)]
    oT = [singles.tile([128, S], BF16, tag=f"oT{i}", name=f"oT{i}") for i in range(4)]

    # ---- pools --------------------------------------------------------------
    xt_pool = ctx.enter_context(tc.tile_pool(name="xt", bufs=2))
    tmp_pool = ctx.enter_context(tc.tile_pool(name="tmp", bufs=2))
    pt_pool = ctx.enter_context(tc.tile_pool(name="pt", bufs=4))
    norm_pool = ctx.enter_context(tc.tile_pool(name="norm", bufs=2))
    ost_pool = ctx.enter_context(tc.tile_pool(name="ost", bufs=2))
    proj_ps = ctx.enter_context(tc.tile_pool(name="proj_ps", bufs=3, space="PSUM"))
    sc_ps = ctx.enter_context(tc.tile_pool(name="sc_ps", bufs=1, space="PSUM"))
    po_ps = ctx.enter_context(tc.tile_pool(name="po_ps", bufs=2, space="PSUM"))
    out_ps = ctx.enter_context(tc.tile_pool(name="out_ps", bufs=1, space="PSUM"))

    def proj_rope(dst, w_sb, xt, ot, sc):
        ps = proj_ps.tile([128, 512], F32, tag="ps", name="ps")
        for ic in range(8):
            nc.tensor.matmul(ps[:], w_sb[ic][:, ot * 128:(ot + 1) * 128],
                             xt[ic][:], start=(ic == 0), stop=(ic == 7))
        ssl = slice(sc * 512, (sc + 1) * 512)
        qsh = tmp_pool.tile([128, 512], F32, tag="qsh", name="qsh")
        nc.vector.stream_shuffle(qsh[:], ps[:], SWAP_MASK)
        t1 = tmp_pool.tile([128, 512], F32, tag="t1", name="t1")
        nc.vector.tensor_tensor(t1[:], ps[:], cosI[:, ssl], mybir.AluOpType.mult)
        t2 = tmp_pool.tile([128, 512], F32, tag="t2", name="t2")
        nc.vector.tensor_tensor(t2[:], qsh[:], sinA[:, ssl], mybir.AluOpType.mult)
        nc.vector.tensor_tensor(dst[ot][:, ssl], t1[:], t2[:], mybir.AluOpType.add)

    for sc in range(4):
        ssl = slice(sc * 512, (sc + 1) * 512)
        xt = []
        for ic in range(8):
            t = xt_pool.tile([128, 512], BF16, tag=f"xt{ic}", name=f"xt{ic}")
            nc.sync.dma_start(out=t, in_=xT[ic * 128:(ic + 1) * 128, ssl])
            xt.append(t)
        for ot in range(4):
            proj_rope(qT, wq_sb, xt, ot, sc)
            proj_rope(kT, wk_sb, xt, ot, sc)
        # v projection for the 4 s-tiles of this chunk
        for stl in range(4):
            st = 4 * sc + stl
            psv = proj_ps.tile([128, 512], F32, tag="ps", name="psv")
            for ic in range(8):
                nc.tensor.matmul(psv[:], xt[ic][:, stl * 128:(stl + 1) * 128],
                                 wv_sb[ic][:], start=(ic == 0), stop=(ic == 7))
            nc.gpsimd.memset(vt[st][:], 1.0)
            # head h dims at columns h*65 + [0..64); column h*65+64 stays 1.0
            v3 = vt[st].rearrange("p (h c) -> p h c", h=8)
            p3 = psv.rearrange("p (h c) -> p h c", h=8)
            nc.vector.tensor_copy(out=v3[:, :, 0:64], in_=p3[:, :, :])

        # ---- attention for query chunk qc = sc ------------------------------
        qc = sc
        qsl = slice(qc * 512, (qc + 1) * 512)
        nkt = 4 * qc + 4
        for hp in range(4):
            po0 = po_ps.tile([128, 512], F32, tag="po", name="po0")
            po1 = po_ps.tile([128, 512], F32, tag="po", name="po1")
            for kt in range(nkt):
                ksl = slice(kt * 128, (kt + 1) * 128)
                ps2 = sc_ps.tile([128, 1024], F32, tag="ps2", name="ps2")
                nc.tensor.matmul(ps2[:, 0:512], kT[hp][0:64, ksl],
                                 qT[hp][0:64, qsl], start=True, stop=True)
                nc.tensor.matmul(ps2[:, 512:1024], kT[hp][64:128, ksl],
                                 qT[hp][64:128, qsl], start=True, stop=True)
                pt = pt_pool.tile([128, 1024], BF16, tag="pt", name="pt")
                nc.scalar.activation(pt[:], ps2[:], mybir.ActivationFunctionType.Exp,
                                     scale=0.125)
                if dbg_aps is not None and qc == 0 and hp == 0 and kt == 0:
                    dcp = tmp_pool.tile([128, 1024], F32, tag="dcp", name="dcp")
                    nc.vector.tensor_copy(out=dcp[:], in_=ps2[:])
                    nc.sync.dma_start(out=dbg_aps["dps2"], in_=dcp[:])
                    nc.sync.dma_start(out=dbg_aps["dpt"], in_=pt[:])
                d = kt - 4 * qc
                if d >= 0:
                    msl = slice(d * 512, (d + 1) * 512)
                    nc.vector.tensor_tensor(pt[:, 0:512], pt[:, 0:512], maskb[:, msl],
                                            mybir.AluOpType.mult)
                    nc.vector.tensor_tensor(pt[:, 512:1024], pt[:, 512:1024],
                                            maskb[:, msl], mybir.AluOpType.mult)
                c0 = (2 * hp) * 65
                c1 = (2 * hp + 1) * 65
                nc.tensor.matmul(po0[0:65, :], vt[kt][:, c0:c0 + 65], pt[:, 0:512],
                                 start=(kt == 0), stop=(kt == nkt - 1))
                nc.tensor.matmul(po1[0:65, :], vt[kt][:, c1:c1 + 65], pt[:, 512:1024],
                                 start=(kt == 0), stop=(kt == nkt - 1))
            if dbg_aps is not None and qc == 0 and hp == 0:
                dcq = tmp_pool.tile([128, 512], F32, tag="dcq", name="dcq")
                nc.vector.tensor_copy(out=dcq[:], in_=po0[:])
                nc.sync.dma_start(out=dbg_aps["dpo0"], in_=dcq[:])
            # normalize: l sits in row 64 of each accumulator.  HW
            # partition_broadcast reads partition 0 only, so shift the
            # reciprocal row 64 -> 0 with a small SBUF->SBUF DMA first.
            lt0 = norm_pool.tile([128, 512], F32, tag="lt", name="lt0")
            nc.vector.reciprocal(lt0[64:65, :], po0[64:65, :])
            lb0 = norm_pool.tile([128, 512], F32, tag="lb", name="lb0")
            nc.sync.dma_start(out=lb0[0:1, :], in_=lt0[64:65, :])
            nc.gpsimd.partition_broadcast(lb0[0:64, :], lb0[0:1, :], 64)
            nc.vector.tensor_tensor(oT[hp][0:64, qsl], po0[0:64, :], lb0[0:64, :],
                                    mybir.AluOpType.mult)
            lt1 = norm_pool.tile([128, 512], F32, tag="lt", name="lt1")
            nc.vector.reciprocal(lt1[64:65, :], po1[64:65, :])
            lb1 = norm_pool.tile([128, 512], F32, tag="lb", name="lb1")
            nc.sync.dma_start(out=lb1[0:1, :], in_=lt1[64:65, :])
            nc.gpsimd.partition_broadcast(lb1[0:64, :], lb1[0:1, :], 64)
            o1 = norm_pool.tile([128, 512], BF16, tag="o1", name="o1")
            nc.vector.tensor_tensor(o1[0:64, :], po1[0:64, :], lb1[0:64, :],
                                    mybir.AluOpType.mult)
            # partition-shifted placement (rows 64:128) via SBUF->SBUF DMA
            nc.sync.dma_start(out=oT[hp][64:128, qsl], in_=o1[0:64, :])

        # ---- output projection for the s-tiles of this chunk ----------------
        for stl in range(4):
            st = 4 * qc + stl
            stsl = slice(st * 128, (st + 1) * 128)
            ost = ost_pool.tile([128, 1024], F32, tag="ost", name="ost")
            for oc in range(2):
                pso = out_ps.tile([128, 512], F32, tag="pso", name="pso")
                osl = slice(oc * 512, (oc + 1) * 512)
                for hp in range(4):
                    nc.tensor.matmul(pso[:], oT[hp][:, stsl], wo_sb[hp][:, osl],
                                     start=(hp == 0), stop=(hp == 3))
                nc.vector.tensor_copy(out=ost[:, osl], in_=pso[:])
            nc.sync.dma_start(out=outp[stsl, :], in_=ost[:])

    if dbg_aps is not None:
        nc.sync.dma_start(out=dbg_aps["dq0"], in_=qT[0][:])
        nc.sync.dma_start(out=dbg_aps["dk0"], in_=kT[0][:])
        nc.sync.dma_start(out=dbg_aps["dv0"], in_=vt[0][:])
        nc.sync.dma_start(out=dbg_aps["do0"], in_=oT[0][:])
        nc.sync.dma_start(out=dbg_aps["do1"], in_=oT[1][:])
        nc.sync.dma_start(out=dbg_aps["do2"], in_=oT[2][:])
        nc.sync.dma_start(out=dbg_aps["do3"], in_=oT[3][:])
        nc.sync.dma_start(out=dbg_aps["dsin"], in_=sinI[:])
        nc.sync.dma_start(out=dbg_aps["dcos"], in_=cosI[:])

    ctx.close()


_NC_CACHE = []
LAST_RESULT = None


def _get_program():
    if not _NC_CACHE:
        _NC_CACHE.append(_build_program())
    return _NC_CACHE[0]


def _host_constants():
    p = np.arange(128)
    invf = (THETA ** (-2.0 * ((p % 64) // 2) / DH)).astype(np.float32)[:, None]
    altsign = np.where(p % 2 == 0, -1.0, 1.0).astype(np.float32)[:, None]
    mask = np.zeros((128, 4 * 512), np.float32)
    fq = np.arange(512)
    for d in range(4):
        mask[:, d * 512:(d + 1) * 512] = (p[:, None] <= fq[None, :] - 128 * d)
    return invf, altsign, mask


def _bf16(a):
    import ml_dtypes
    return np.ascontiguousarray(a).astype(ml_dtypes.bfloat16)


def kernel(x, token_positions, wq, wk, wv, wo):
    x = np.asarray(x, dtype=np.float32)
    pos = np.asarray(token_positions, dtype=np.int32)
    wq = np.asarray(wq, dtype=np.float32)
    wk = np.asarray(wk, dtype=np.float32)
    wv = np.asarray(wv, dtype=np.float32)
    wo = np.asarray(wo, dtype=np.float32)

    nc = _get_program()
    invf, altsign, mask = _host_constants()

    in_maps = []
    for c in range(8):
        b, g = c // 2, c % 2
        gsl = slice(g * GD, (g + 1) * GD)
        in_maps.append({
            "xT": _bf16(x[b].T),
            "wqT": _bf16(wq.T[:, gsl]),
            "wkT": _bf16(wk.T[:, gsl]),
            "wvT": _bf16(wv.T[:, gsl]),
            "woT": _bf16(wo.T[gsl, :]),
            "pos": pos[None, :].copy(),
            "invf": invf,
            "altsign": altsign,
            "mask01": mask,
        })

    old_m = nc.m
    nc.m = get_hw_module(nc.m)
    try:
        res = run_bass_kernel_spmd(nc, in_maps, core_ids=list(range(8)))
    finally:
        nc.m = old_m
    global LAST_RESULT
    LAST_RESULT = res

    out = np.empty((B, S, D), dtype=np.float32)
    for b in range(B):
        # tensor-parallel gather: sum the two head-group partials per batch
        out[b] = res.results[2 * b]["outp"] + res.results[2 * b + 1]["outp"]
    return out
